# revision 26
# baseline (speedup 1.0000x reference)
"""Dilated segment attention on 8 Trainium2 NeuronCores (Bass/Tile).

Problem: x:[4,8192,1024] fp32. Per 64-token segment, rows ::2 are kept
(32 tokens), projected with Wq/Wk/Wv (+bias), and full-dim attention is
computed within each segment. Output: [4,4096,1024] fp32.

Sharding: data-parallel. Core c handles batch c//2, sequence half c%2 ->
2048 dilated tokens = 64 segments. No collectives.

Key algebraic restructure: softmax over keys cancels every term of
q_j.k_i that is constant in the key index i, so

  softmax_i(q_j . k_i) = softmax_i( x_j A x_i^T + x_i . w ),
  A = Wq^T Wk,  w = bq Wk        (bk drops out entirely).

The kernel computes A on-device once per core (1024^3 MACs, started as
soon as the 4 MB of Wq/Wk bf16 lands) and replaces BOTH the q and k
projections with a single h-projection h = x A + w; simT[i,j] = x_i.h_j.
This cuts projection matmul work from 3 passes to 2 (plus the cheap A).

Host prep is layout/dtype only: weights passed bf16 (native Wq/Wk for
the A matmul, Wv^T for the v pass), x passed dilated+transposed+bf16 in
chunk-major [4*1024, 512] layout. All SBUF residents use single big
tiles so each load is ONE 1-2 MB DMA op (descriptor-efficient), halves
split across the two HWDGE rings (sync + scalar) in priority order
wk -> wq -> x chunk 0 / wv -> x chunks 1-3.

Per-core pipeline (all matmuls bf16, fp32 PSUM):
  - warm-up junk matmuls keep the PE HAM at 2.4 GHz until real work
  - w = bq Wk (64 tiny matmuls, needs only Wk), A = Wq^T Wk (128
    matmuls, N=512) -> A in SBUF bf16
  - per chunk c: v-pass (x stationary, Wv^T moving); h-pass (A
    stationary, x.T moving) with w fused into the ACT psum->sbuf
    epilogue; simT per 4-segment group as one packed 128x128 matmul
    over 8 k-tiles plus a rank-4 mask matmul that puts -30000 on the
    off-diagonal 32x32 blocks so one full-tile ACT exp yields the
    block-diagonal p (off-diag underflows to exactly 0); attn@v and the
    softmax denominator l (ones-column matmul) per token tile; final
    out = psum_av * (1/l) + bv in one DVE scalar_tensor_tensor, written
    out on the (by then idle) HWDGE rings.
"""

import numpy as np

P = 128
D = 1024
KT = 8    # d tiles of 128
OT = 8    # d_out tiles of 128
NTT = 16  # token tiles of 128 (2048 tokens per core)
FD = 512  # matmul moving free dim / psum bank
TCH = 4   # token chunks of 512
NEG = -30000.0  # off-diagonal mask; exp(scale*(sim+NEG)) underflows to 0

_CACHE = {}


def _build_nc():
    import os
    from contextlib import ExitStack

    import concourse.bass as bass
    import concourse.mybir as mybir
    import concourse.tile as tile
    from concourse import bacc

    KWARM = int(os.environ.get("KWARM", "40"))

    dt = mybir.dt
    AF = mybir.ActivationFunctionType
    ALU = mybir.AluOpType

    nc = bacc.Bacc("TRN2", target_bir_lowering=False, debug=False,
                   enable_asserts=False)

    # x.T, dilated, bf16, chunk-major: row 1024*c + d holds x.T[d, 512c:...]
    xt_d = nc.dram_tensor("xt", [TCH * D, FD], dt.bfloat16,
                          kind="ExternalInput")
    wq_d = nc.dram_tensor("wqn", [D, D], dt.bfloat16, kind="ExternalInput")
    wk_d = nc.dram_tensor("wkn", [D, D], dt.bfloat16, kind="ExternalInput")
    wv_d = nc.dram_tensor("wvt", [D, D], dt.bfloat16, kind="ExternalInput")
    bqc_d = nc.dram_tensor("bqc", [P, KT], dt.bfloat16, kind="ExternalInput")
    bvb_d = nc.dram_tensor("bvb", [1, D], dt.bfloat16, kind="ExternalInput")
    mskl_d = nc.dram_tensor("mskl", [4, P], dt.bfloat16, kind="ExternalInput")
    mskr_d = nc.dram_tensor("mskr", [4, P], dt.bfloat16, kind="ExternalInput")
    out_d = nc.dram_tensor("out", [2048, D], dt.float32, kind="ExternalOutput")

    scale = float(D) ** -0.5

    with tile.TileContext(nc) as tc, ExitStack() as ctx:
        consts = ctx.enter_context(tc.tile_pool(name="consts", bufs=1))
        resid = ctx.enter_context(tc.tile_pool(name="resid", bufs=1))
        outp = ctx.enter_context(tc.tile_pool(name="outp", bufs=3))
        rsbp = ctx.enter_context(tc.tile_pool(name="rsbp", bufs=2))

        ones_col = consts.tile([P, 1], dt.bfloat16, name="ones_col")
        ones_row = consts.tile([1, P], dt.bfloat16, name="ones_row")
        maskL = consts.tile([4, P], dt.bfloat16, name="maskL")
        maskR = consts.tile([4, P], dt.bfloat16, name="maskR")
        junk_w = consts.tile([P, P], dt.bfloat16, name="junk_w")
        junk_m = consts.tile([P, P], dt.bfloat16, name="junk_m")
        bqc_sb = consts.tile([P, KT], dt.bfloat16, name="bqc_sb")
        bvb_sb = consts.tile([1, D], dt.bfloat16, name="bvb_sb")
        w_sb = consts.tile([P, OT], dt.float32, name="w_sb")
        bv_rep = consts.tile([P, D], dt.float32, name="bv_rep")

        nc.vector.memset(ones_col[:], 1.0)
        nc.vector.memset(ones_row[:], 1.0)
        nc.vector.memset(junk_w[:], 0.0)
        nc.vector.memset(junk_m[:], 0.0)

        # big SBUF residents; each DMA below is one 1-2 MB op
        xTall = resid.tile([P, TCH * KT * FD], dt.bfloat16, name="xTall")
        wkall = resid.tile([P, KT * D], dt.bfloat16, name="wkall")
        wqall = resid.tile([P, KT * D], dt.bfloat16, name="wqall")
        wvall = resid.tile([P, KT * D], dt.bfloat16, name="wvall")
        A_sb = [resid.tile([P, D], dt.bfloat16, name=f"A{m}")
                for m in range(KT)]
        hT = [resid.tile([P, 2048], dt.bfloat16, name=f"hT{o}")
              for o in range(OT)]
        vv = [resid.tile([P, D], dt.bfloat16, name=f"v{t}") for t in range(NTT)]
        pT = [resid.tile([P, P], dt.bfloat16, name=f"pT{g}") for g in range(NTT)]

        def xT(k, lo, n):
            """columns [lo, lo+n) of d-stripe k of x.T (token index)."""
            c, j = divmod(lo, FD)
            assert j + n <= FD
            base = (KT * c + k) * FD + j
            return xTall[:, base:base + n]

        def wsl(wall, i, lo, n):
            return wall[:, D * i + lo:D * i + lo + n]

        # ---- DMA priority order. Tiny consts ride the idle SWDGE ring.
        # Wk/Wq go in 2-tile (256 KB) pieces, interleaved across the two
        # HWDGE rings in i order, so the i-outer A sweep starts as soon as
        # the first pair lands.  Then wv (scalar) and x chunks (sync).
        nc.gpsimd.dma_start(bqc_sb[:], bqc_d[:])
        nc.gpsimd.dma_start(bvb_sb[:], bvb_d[:])
        nc.gpsimd.dma_start(maskL[:], mskl_d[:])
        nc.gpsimd.dma_start(maskR[:], mskr_d[:])

        def load_w_tiles(eng, wall, w_d, lo, n):
            src = bass.AP(w_d, lo * P * D, [[D, P], [P * D, n], [1, D]])
            eng.dma_start(wall[:, lo * D:(lo + n) * D], src)

        def load_w_cols(eng, wall, w_d, i, lo, n):
            src = bass.AP(w_d, i * P * D + lo, [[D, P], [1, n]])
            eng.dma_start(wall[:, i * D + lo:i * D + lo + n], src)

        # single-tile pieces throughout: the i-outer A sweep is gated
        # per-i, so each 256 KB arrival unlocks the next 8 matmuls.  The
        # very first tile of each weight goes in two 128 KB column-halves
        # so the sweep's first matmuls gate on as little DMA as possible.
        for hp in range(2):
            load_w_cols(nc.scalar, wkall, wk_d, 0, hp * FD, FD)
            load_w_cols(nc.sync, wqall, wq_d, 0, hp * FD, FD)
        for i in range(1, KT):
            load_w_tiles(nc.scalar, wkall, wk_d, i, 1)
            load_w_tiles(nc.sync, wqall, wq_d, i, 1)
        src = bass.AP(wv_d, 0, [[D, P], [P * D, KT], [1, D]])
        nc.scalar.dma_start(wvall[:], src)
        for c in range(TCH):
            src = bass.AP(xt_d, c * D * FD, [[FD, P], [P * FD, KT], [1, FD]])
            nc.sync.dma_start(xTall[:, KT * FD * c:KT * FD * (c + 1)], src)

        with tc.tile_pool(name="projp", bufs=3, space="PSUM") as projp, \
             tc.tile_pool(name="simp", bufs=2, space="PSUM") as simp, \
             tc.tile_pool(name="avp", bufs=2, space="PSUM") as avp, \
             tc.tile_pool(name="lp", bufs=1, space="PSUM") as lp:

            # ---- HAM warm-up: junk matmuls from t~0 so the PE clock is
            # at 2.4 GHz when the first real matmuls arrive.
            if KWARM:
                wps = projp.tile([P, FD], dt.float32, name="pps")
                for _ in range(KWARM):
                    nc.tensor.matmul(wps[:, 0:P], junk_w[:], junk_m[:],
                                     start=True, stop=True)

            # ---- A = Wq^T @ Wk  (A[a, b] = sum_o Wq[o, a] Wk[o, b]).
            # i-OUTER with 8 simultaneous accumulators (one psum bank each,
            # borrowed across all four pools = exactly 8 banks): the first
            # 4 i-steps contract the first halves of Wq/Wk, so the sweep
            # starts as soon as those 2 MB land instead of waiting for all
            # 4 MB.  w = bq @ Wk slots between the two half-sweeps, giving
            # the DVE time to evacuate sweep 0 before its banks are reused.
            def a_sweep(hf):
                acc = [projp.tile([P, FD], dt.float32, name="pps")
                       for _ in range(3)]
                acc += [simp.tile([P, FD], dt.float32, name="sps")
                        for _ in range(2)]
                acc += [avp.tile([P, FD], dt.float32, name="avs")
                        for _ in range(2)]
                acc += [lp.tile([P, FD], dt.float32, name="lps")]
                for i in range(KT):
                    for m in range(KT):
                        nc.tensor.matmul(acc[m][:],
                                         wsl(wqall, i, P * m, P),
                                         wsl(wkall, i, FD * hf, FD),
                                         start=(i == 0), stop=(i == KT - 1))
                for m in range(KT):
                    nc.vector.tensor_copy(A_sb[m][:, FD * hf:FD * hf + FD],
                                          acc[m][:])

            # Sweep 0 (hf=0) also folds in w = bq @ Wk: w's tiny N=1
            # matmuls gate on the same wk singles and hide between the
            # N=512 A matmuls, keeping PE duty high enough that the HAM
            # never re-throttles (a standalone w phase measured ~40% duty
            # and triggered a MID re-throttle).  w_ps takes the 9th psum
            # bank (lp), so this sweep runs 7 accumulators (m=0..6) plus a
            # trailing m=7 group.  w_ps is zero-initialized by one K=128
            # matmul against the zero tile — that start=True sets
            # has_written for its 8 columns exactly once, so the
            # interleaved per-column accumulations all use start=False.
            w_ps = lp.tile([P, OT], dt.float32, name="lps")
            nc.tensor.matmul(w_ps[:], junk_w[:], bqc_sb[:],
                             start=True, stop=False)
            acc = [projp.tile([P, FD], dt.float32, name="pps")
                   for _ in range(3)]
            acc += [simp.tile([P, FD], dt.float32, name="sps")
                    for _ in range(2)]
            acc += [avp.tile([P, FD], dt.float32, name="avs")
                    for _ in range(2)]
            for i in range(KT):
                for m in range(7):
                    nc.tensor.matmul(acc[m][:], wsl(wqall, i, P * m, P),
                                     wsl(wkall, i, 0, FD),
                                     start=(i == 0), stop=(i == KT - 1))
                for m in range(OT):
                    nc.tensor.matmul(w_ps[:, m:m + 1],
                                     wsl(wkall, i, P * m, P),
                                     bqc_sb[:, i:i + 1],
                                     start=False, stop=(i == KT - 1))
            for m in range(7):
                nc.vector.tensor_copy(A_sb[m][:, 0:FD], acc[m][:])
            psA = projp.tile([P, FD], dt.float32, name="pps")
            for i in range(KT):
                nc.tensor.matmul(psA[:], wsl(wqall, i, P * 7, P),
                                 wsl(wkall, i, 0, FD),
                                 start=(i == 0), stop=(i == KT - 1))
            nc.vector.tensor_copy(A_sb[7][:, 0:FD], psA[:])
            nc.vector.tensor_copy(w_sb[:], w_ps[:])

            # ---- bv broadcast to all partitions via K=1 ones matmul
            for dh in range(2):
                ps = projp.tile([P, FD], dt.float32, name="pps")
                nc.tensor.matmul(ps[:], ones_row[:],
                                 bvb_sb[:, FD * dh:FD * dh + FD],
                                 start=True, stop=True)
                nc.scalar.copy(bv_rep[:, FD * dh:FD * dh + FD], ps[:])

            a_sweep(1)

            def v_pass(t):
                pss = [projp.tile([P, FD], dt.float32, name="pps")
                       for _ in range(2)]
                for i in range(KT):
                    for dh in range(2):
                        nc.tensor.matmul(pss[dh][:],
                                         xT(i, P * t, P),
                                         wsl(wvall, i, FD * dh, FD),
                                         start=(i == 0), stop=(i == KT - 1))
                for dh in range(2):
                    nc.vector.tensor_copy(vv[t][:, FD * dh:FD * dh + FD],
                                          pss[dh][:])

            def av_group(t, eng):
                lps = lp.tile([P, 1], dt.float32, name="lps")
                nc.tensor.matmul(lps[:], pT[t][:], ones_col[:],
                                 start=True, stop=True)
                rsb = rsbp.tile([P, 1], dt.float32, name="rsb")
                nc.vector.reciprocal(rsb[:], lps[:])
                osb = outp.tile([P, D], dt.float32, name="osb")
                for dh in range(2):
                    avs = avp.tile([P, FD], dt.float32, name="avs")
                    nc.tensor.matmul(avs[:], pT[t][:],
                                     vv[t][:, FD * dh:FD * dh + FD],
                                     start=True, stop=True)
                    nc.vector.scalar_tensor_tensor(
                        osb[:, FD * dh:FD * dh + FD], avs[:], rsb[:],
                        bv_rep[:, FD * dh:FD * dh + FD],
                        ALU.mult, ALU.add)
                    eng.dma_start(
                        bass.AP(out_d, t * P * D + FD * dh,
                                [[D, P], [1, FD]]),
                        osb[:, FD * dh:FD * dh + FD])

            for c in range(TCH):
                # v for this chunk (needs only x chunk c + Wv).  The last
                # chunk's v tiles are instead interleaved with its attn@v
                # below, so the final STT/out epilogues hide behind v
                # matmuls instead of bunching DVE-bound at the very end.
                if c < TCH - 1:
                    for t in range(4 * c, 4 * c + 4):
                        v_pass(t)
                # h-pass for chunk c: h = x A + w, stored transposed
                for o in range(OT):
                    psH = projp.tile([P, FD], dt.float32, name="pps")
                    for i in range(KT):
                        nc.tensor.matmul(psH[:],
                                         A_sb[i][:, P * o:P * o + P],
                                         xT(i, FD * c, FD),
                                         start=(i == 0), stop=(i == KT - 1))
                    nc.scalar.activation(hT[o][:, FD * c:FD * c + FD],
                                         psH[:], AF.Identity,
                                         bias=w_sb[:, o:o + 1], scale=1.0)
                # simT + exp for this chunk's 4 groups
                for g in range(4 * c, 4 * c + 4):
                    sps = simp.tile([P, P], dt.float32, name="sps")
                    nc.tensor.matmul(sps[:], maskL[:], maskR[:],
                                     start=True, stop=False)
                    for kk in range(KT):
                        nc.tensor.matmul(sps[:],
                                         xT(kk, P * g, P),
                                         hT[kk][:, P * g:P * g + P],
                                         start=False, stop=(kk == KT - 1))
                    nc.scalar.activation(pT[g][:], sps[:], AF.Exp,
                                         bias=0.0, scale=scale)
                # attn@v for this chunk; out rides the idle HWDGE rings
                for t in range(4 * c, 4 * c + 4):
                    if c == TCH - 1:
                        v_pass(t)
                    av_group(t, nc.sync if t % 2 == 0 else nc.scalar)

    nc.compile()
    return nc


def get_nc():
    if "nc" not in _CACHE:
        _CACHE["nc"] = _build_nc()
    return _CACHE["nc"]


def make_in_maps(x, Wq, bq, Wk, bk, Wv, bv):
    import ml_dtypes

    bf16 = ml_dtypes.bfloat16
    x = np.asarray(x, np.float32)
    wqn = np.ascontiguousarray(np.asarray(Wq, np.float32).astype(bf16))
    wkn = np.ascontiguousarray(np.asarray(Wk, np.float32).astype(bf16))
    wvt = np.ascontiguousarray(np.asarray(Wv, np.float32).astype(bf16).T)
    bqc = np.ascontiguousarray(
        np.asarray(bq, np.float32).astype(bf16).reshape(KT, P).T)
    bvb = np.asarray(bv, np.float32).reshape(1, D).astype(bf16)
    # maskL[a, i] = 1 iff i in 32-block a; maskR[a, j] = NEG unless j in a
    blk = (np.arange(P) // 32)[None, :] == np.arange(4)[:, None]
    mskl = blk.astype(bf16)
    mskr = np.where(blk, 0.0, NEG).astype(bf16)
    in_maps = []
    for c in range(8):
        b, h = divmod(c, 2)
        xs = x[b, 4096 * h:4096 * h + 4096]
        xs = xs.reshape(64, 64, D)[:, ::2, :].reshape(2048, D).astype(bf16)
        # x.T in chunk-major rows: row 1024*c + d = x.T[d, 512c:512c+512]
        xt = np.ascontiguousarray(
            xs.T.reshape(D, TCH, FD).transpose(1, 0, 2)).reshape(TCH * D, FD)
        in_maps.append({"xt": xt, "wqn": wqn, "wkn": wkn, "wvt": wvt,
                        "bqc": bqc, "bvb": bvb, "mskl": mskl, "mskr": mskr})
    return in_maps


def kernel(x, Wq, bq, Wk, bk, Wv, bv):
    from concourse.bass_utils import run_bass_kernel_spmd

    nc = get_nc()
    in_maps = make_in_maps(x, Wq, bq, Wk, bk, Wv, bv)
    try:
        res = run_bass_kernel_spmd(nc, in_maps, core_ids=list(range(8)))
    except Exception:
        # one retry against a transiently wedged device
        res = run_bass_kernel_spmd(nc, in_maps, core_ids=list(range(8)))
    _CACHE["last_res"] = res
    out = np.empty((4, 4096, D), np.float32)
    for c in range(8):
        b, h = divmod(c, 2)
        out[b, 2048 * h:2048 * h + 2048] = res.results[c]["out"]
    return out
